# revision 12
# baseline (speedup 1.0000x reference)
# Trainium2 Bass kernel for batched CG combine:
#   out[i, p, a, b] = sum_{m,n} A[i, m, a] * B[i, n, b] * C[m, n, p]
# A: (600000, 3, 3) f32, B: (600000, 5, 5) f32, C: (3, 5, 5) f32
# out: (600000, 5, 15) f32
#
# Algorithm: exact rank-8 CP decomposition C[m,n,p] = sum_r U[m,r] V[n,r] W[p,r].
# The host pre-expands the B side:  BV_E[i, q] = sum_n (V[n,r] dirac_b) B[i,(n,b)]
# (q = (r,a,b), 120 rows, fp16) so that on-device, per 1024-atom pair:
#   au[q, i]  = sum_m (U[m,r] dirac_a) A[(m,a), i]     (PE matmul K=9 -> PSUM f32)
#   p         = BV_E (SBUF fp16) * au (PSUM f32)       (ONE wide DVE multiply)
#   out[(p,a,b), i] = WO^T p                           (PE matmul K=120 -> PSUM f32)
#   ost       = copy(out)                              (ONE wide ACT copy -> fp16)
# This removes the PSUM->SBUF copy of au that the previous version needed
# (vector ops accept one PSUM operand, so the DVE multiply can consume au
# directly from PSUM). Per pair only 2 vector-engine ops run (DVE mul, ACT
# ocopy) instead of 3 per 500-atom tile before. All HBM IO is fp16.
#
# TRN2 notes driving the design:
#  - matmul output to PSUM must be fp32 (16-bit PSUM is TRN3+), so every
#    PSUM-source vector op runs at 1x: cost ~ (init + FD) cycles.
#  - PSUM = 8 banks x 2KB: au [120,1024] f32 = 2 banks x2 bufs, o [75,1024]
#    f32 = 2 banks x2 bufs -> exactly 8 banks, double-buffered.
#  - Wide (FD=1024, 2-bank contiguous) ops amortize the ~120-170 cycle init
#    + ~250ns dispatch overhead per instruction.
#
# Sharding: data-parallel over atoms, 75000 per core across 8 cores.

import numpy as np

N_ATOMS = 600000
NCORES = 8
NPC = N_ATOMS // NCORES   # 75000
PAIRW = 1024              # atoms per wide op (2 x 512-col PSUM halves)
NPAIRS = 74               # pairs per core
NPAD = NPAIRS * PAIRW     # 75776 padded atoms per core
CH = 2                    # pairs per input DMA chunk
OG = 2                    # pairs per output staging buffer / DMA

R = 8  # CP rank

U = np.array([[0.2419016152442985, 0.6625062831986197, -0.8309374270990885, 0.3998142823675103, -0.5651140448972596, -0.34640840162110975, 0.7646485241540064, -0.0981640650113134], [0.9679329076741274, -0.6672684032643771, -0.5353370910241713, -0.9127024843358726, 0.26799289625560263, 0.8715541794335616, -0.5278177753574712, -0.018552310924435454], [0.06774581008230969, 0.3403502647675755, 0.1515163067782647, -0.08439617705843598, 0.7802729803193187, 0.34697915153247866, 0.3697580702645849, -0.9949973005490104]])
V = np.array([[0.0026140108173807915, 0.6944345633371292, -0.5652773041221544, -0.35343275859595025, -0.03433664562735461, 0.08091670140460634, -0.0892103404240648, -0.1980300231087587], [0.2576248520364635, 0.06539948454957029, -0.35434557927644844, -0.03640441158856663, -0.7413593971475833, 0.0030001701455498278, 0.3713639451526768, 0.016947075929799594], [-0.5377309758940755, -0.02096760544900235, 0.40365084423895436, 0.5095417434602116, -0.45423293309175394, -0.5702820721334585, 0.6190313285414931, 0.7858326418298565], [0.7170730175523563, 0.7001885499108222, 0.4925926570601597, -0.7743826610421906, -0.16559112080190702, 0.6571136713106263, -0.6611900442465742, -0.2983796128216165], [0.36093529561820403, -0.15093011216763902, -0.38641849081949886, 0.1202443758222842, -0.4641758957921707, -0.4862339638412094, 0.1837342512310362, 0.5039182198056593]])
W = np.array([[0.7951356712114984, -0.07784905999497176, 0.08450253790371903, 0.006843070854248517, 0.2048617974624018, -1.523924051439455, 0.8830139483275325, 0.5211882387254724], [0.5093941381116157, -0.7659769028241413, -0.3653038243879763, -0.8496149079844891, 0.052715213787387104, 0.18251310702150852, 0.268561851999145, 0.9142889507799132], [0.021385010903070902, -0.4182776710107811, 0.26977388961992294, -1.1442626505742266, -1.0048448949104412, 0.34663597211489194, 1.2092826345430325, 0.8086175923533013], [-0.9015995943490751, 1.249123426342828, -0.5049639898080718, 2.545125440023137, 0.16782025096354364, -1.5011481522860137, 0.409842324079843, 0.27493076503176855], [0.9934580335307789, -0.10023212966102599, -0.4889278808326145, -2.6183798202363553, -0.4522780676075401, 1.1697194808175109, 0.8428489593111734, 0.2161166285673376]])


def _cp_factors_for(C):
    """Return (U, V, W) float64 with C[m,n,p] ~= sum_r U[m,r]V[n,r]W[p,r].

    Uses the embedded factors when C matches their reconstruction (the fixed
    real-CG tensor for l1=1, l2=2, L=2); otherwise fits a rank-8 CP
    decomposition to the given C at runtime via ALS with restarts.
    """
    C = np.asarray(C, dtype=np.float64)
    recon = np.einsum('mr,nr,pr->mnp', U, V, W)
    if np.abs(recon - C).max() < 1e-5 * max(1.0, np.abs(C).max()):
        return U, V, W

    def khatri(X, Y):
        return (X[:, None, :] * Y[None, :, :]).reshape(-1, X.shape[1])

    C1 = C.reshape(3, 25)
    C2 = C.transpose(1, 0, 2).reshape(5, 15)
    C3 = C.transpose(2, 0, 1).reshape(5, 15)
    best = None
    for seed in range(64):
        rng = np.random.default_rng(seed)
        u = rng.standard_normal((3, R))
        v = rng.standard_normal((5, R))
        w = rng.standard_normal((5, R))
        for _ in range(3000):
            u = C1 @ np.linalg.pinv(khatri(v, w).T)
            v = C2 @ np.linalg.pinv(khatri(u, w).T)
            w = C3 @ np.linalg.pinv(khatri(u, v).T)
        err = np.abs(np.einsum('mr,nr,pr->mnp', u, v, w) - C).max()
        if best is None or err < best[0]:
            best = (err, u, v, w)
        if err < 1e-9 * max(1.0, np.abs(C).max()):
            break
    err, u, v, w = best
    if err > 1e-5 * max(1.0, np.abs(C).max()):
        raise RuntimeError(f"runtime CP fit of C failed: absmax err {err}")
    su = np.linalg.norm(u, axis=0)
    sv = np.linalg.norm(v, axis=0)
    return u / su, v / sv, w * (su * sv)


def _build_weights(u, v, w):
    """WA [9,120], WB40 [25,40], WO [120,75] f32; q = a*40 + r*5 + b.

    The q ordering groups the three a-blocks contiguously so the device can
    build the 120-row BV_E operand by replicating one 40-row (r,b) block
    three times with on-chip SBUF->SBUF DMA (HBM ships only 40 rows).
    """
    WA = np.zeros((9, 15 * R), np.float32)
    WB40 = np.zeros((25, 5 * R), np.float32)
    WO = np.zeros((15 * R, 75), np.float32)
    for r in range(R):
        for a in range(3):
            for b in range(5):
                q = a * 40 + r * 5 + b
                for m in range(3):
                    WA[m * 3 + a, q] = u[m, r]
                for p in range(5):
                    WO[q, p * 15 + a * 5 + b] = w[p, r]
    for r in range(R):
        for b in range(5):
            for n in range(5):
                WB40[n * 5 + b, r * 5 + b] = v[n, r]
    return WA, WB40, WO


def _build_nc(WA, WO, reps=1):
    import concourse.bass as bass
    import concourse.bacc as bacc
    import concourse.mybir as mybir
    from concourse import tile

    f32 = mybir.dt.float32
    f16 = mybir.dt.float16

    nc = bacc.Bacc()
    a_in = nc.declare_dram_parameter("a_pack", [9, NPAD], f16, isOutput=False)
    bv_in = nc.declare_dram_parameter("bv_pack", [5 * R, NPAD], f16,
                                      isOutput=False)
    out_d = nc.declare_dram_parameter("out_t", [75, NPAD], f16, isOutput=True)
    wa_d = nc.inline_tensor(WA.astype(np.float16), name="wa")
    wo_d = nc.inline_tensor(WO.astype(np.float16), name="wo")

    with tile.TileContext(nc) as tc:
        with (
            tc.tile_pool(name="const", bufs=1) as cpool,
            tc.tile_pool(name="a", bufs=3) as a_pool,
            tc.tile_pool(name="bv", bufs=3) as bv_pool,
            tc.tile_pool(name="p", bufs=3) as p_pool,
            tc.tile_pool(name="ost", bufs=2) as ost_pool,
            tc.tile_pool(name="au_ps", bufs=2, space=bass.MemorySpace.PSUM) as au_ps,
            tc.tile_pool(name="o_ps", bufs=2, space=bass.MemorySpace.PSUM) as o_ps,
        ):
            wa_t = cpool.tile([9, 15 * R], f16, tag="wa")
            wo_t = cpool.tile([15 * R, 75], f16, tag="wo")
            nc.gpsimd.dma_start(wa_t[:], wa_d[:, :])
            nc.gpsimd.dma_start(wo_t[:], wo_d[:, :])

            import contextlib
            rep_ctx = (tc.For_i(0, reps, 1) if reps > 1
                       else contextlib.nullcontext())
            with rep_ctx:
                # Software-pipelined by one pair: issue pair t's au-matmuls
                # BEFORE pair t-1's o-matmuls so the (in-order) PE queue
                # computes au(t) while the DVE multiply of pair t-1 runs.
                # Without this the PE's o-mm(t-1) [which waits on mul(t-1)]
                # blocks au-mm(t), serializing DVE and PE each pair.
                ost = None
                prev = None
                nchunks = NPAIRS // CH
                chunk_cache = {}

                def load_chunk(k):
                    cw = CH * PAIRW
                    a_t = a_pool.tile([9, cw], f16, tag="a")
                    nc.sync.dma_start(a_t[:], a_in[:, k * cw:(k + 1) * cw])
                    # HBM ships the 40 distinct (r,b) rows; replicate the
                    # a-blocks on-chip (SBUF->SBUF DMA, no HBM traffic).
                    bv_t = bv_pool.tile([15 * R, cw], f16, tag="bv")
                    nc.sync.dma_start(bv_t[0:40], bv_in[:, k * cw:(k + 1) * cw])
                    nc.sync.dma_start(bv_t[40:80], bv_t[0:40])
                    nc.sync.dma_start(bv_t[80:120], bv_t[0:40])
                    chunk_cache[k] = (a_t, bv_t)

                for t in range(NPAIRS + 1):
                    if t < NPAIRS:
                        k, j = divmod(t, CH)
                        # prefetch one chunk ahead of use
                        if t == 0:
                            load_chunk(0)
                            if nchunks > 1:
                                load_chunk(1)
                        elif j == 0 and k + 1 < nchunks:
                            load_chunk(k + 1)
                        a_t, bv_t = chunk_cache[k]
                        au = au_ps.tile([15 * R, PAIRW], f32, tag="au")
                        for h in (0, 1):
                            nc.tensor.matmul(
                                au[:, 512 * h:512 * (h + 1)],
                                wa_t[:],
                                a_t[:, j * PAIRW + 512 * h:
                                    j * PAIRW + 512 * (h + 1)],
                                tile_position=(0, 0),
                            )
                        cur = (au, bv_t, j)
                    else:
                        cur = None

                    if prev is not None:
                        au_p, bv_p, jp = prev
                        tp = t - 1
                        p = p_pool.tile([15 * R, PAIRW], f16, tag="p")
                        nc.vector.tensor_mul(
                            p[:], bv_p[:, jp * PAIRW:(jp + 1) * PAIRW],
                            au_p[:])
                        o = o_ps.tile([75, PAIRW], f32, tag="o")
                        for h in (0, 1):
                            nc.tensor.matmul(
                                o[:, 512 * h:512 * (h + 1)],
                                wo_t[:],
                                p[:, 512 * h:512 * (h + 1)],
                                tile_position=(0, 0),
                            )
                        g, gs = divmod(tp, OG)
                        if gs == 0:
                            ost = ost_pool.tile([75, OG * PAIRW], f16,
                                                tag="ost")
                        nc.scalar.copy(
                            ost[:, gs * PAIRW:(gs + 1) * PAIRW], o[:])
                        if gs == OG - 1:
                            # NOTE: issuing this on nc.scalar's HWDGE ring
                            # measured 163us vs 113.8us -- the trigger stalls
                            # the ACT ocopy stream. Keep it on nc.sync.
                            nc.sync.dma_start(
                                out_d[:, OG * PAIRW * g:OG * PAIRW * (g + 1)],
                                ost[:],
                            )
                    prev = cur
    nc.finalize()
    return nc


def _pack_inputs(A, B, WB40):
    """Per-core packed fp16 [9, NPAD] (A) and [40, NPAD] (BV40) arrays."""
    a_maps = []
    bv_maps = []
    BV40 = (B.reshape(N_ATOMS, 25) @ WB40).astype(np.float16)  # [N, 40]
    A16 = A.reshape(N_ATOMS, 9).astype(np.float16)
    for c in range(NCORES):
        Apack = np.zeros((9, NPAD), np.float16)
        Apack[:, :NPC] = A16[c * NPC:(c + 1) * NPC].T
        BVpack = np.zeros((5 * R, NPAD), np.float16)
        BVpack[:, :NPC] = BV40[c * NPC:(c + 1) * NPC].T
        a_maps.append(Apack)
        bv_maps.append(BVpack)
    return a_maps, bv_maps


_NC_CACHE = {}


def kernel(A, B, C):
    from concourse.bass_utils import run_bass_kernel_spmd

    A = np.ascontiguousarray(np.asarray(A, dtype=np.float32))
    B = np.ascontiguousarray(np.asarray(B, dtype=np.float32))
    C = np.asarray(C, dtype=np.float32)

    key = C.tobytes()
    if key not in _NC_CACHE:
        u, v, w = _cp_factors_for(C)
        WA, WB, WO = _build_weights(u, v, w)
        _NC_CACHE[key] = (_build_nc(WA, WO), WB)
    nc, WB = _NC_CACHE[key]

    a_maps, bv_maps = _pack_inputs(A, B, WB)
    in_maps = [{"a_pack": a_maps[c], "bv_pack": bv_maps[c]}
               for c in range(NCORES)]
    res = run_bass_kernel_spmd(nc, in_maps, list(range(NCORES)))
    outs = [res.results[c]["out_t"][:, :NPC] for c in range(NCORES)]
    full = np.concatenate(outs, axis=1).astype(np.float32)  # [75, 600000]
    return np.ascontiguousarray(full.T).reshape(N_ATOMS, 5, 15)


if __name__ == "__main__":
    rng = np.random.default_rng(0)
    A = rng.standard_normal((N_ATOMS, 3, 3)).astype(np.float32)
    B = rng.standard_normal((N_ATOMS, 5, 5)).astype(np.float32)
    C = np.einsum('mr,nr,pr->mnp', U, V, W).astype(np.float32)
    out = kernel(A, B, C)
    print(out.shape, out.dtype)


# revision 17
# speedup vs baseline: 1.2909x; 1.2909x over previous
# Trainium2 Bass kernel for batched CG combine:
#   out[i, p, a, b] = sum_{m,n} A[i, m, a] * B[i, n, b] * C[m, n, p]
# A: (600000, 3, 3) f32, B: (600000, 5, 5) f32, C: (3, 5, 5) f32
# out: (600000, 5, 15) f32
#
# Algorithm: exact rank-8 CP decomposition C[m,n,p] = sum_r U[m,r] V[n,r] W[p,r].
# The host pre-expands the B side:  BV_E[i, q] = sum_n (V[n,r] dirac_b) B[i,(n,b)]
# (q = (r,a,b), 120 rows, fp16) so that on-device, per 1024-atom pair:
#   au[q, i]  = sum_m (U[m,r] dirac_a) A[(m,a), i]     (PE matmul K=9 -> PSUM f32)
#   p         = BV_E (SBUF fp16) * au (PSUM f32)       (ONE wide DVE multiply)
#   out[(p,a,b), i] = WO^T p                           (PE matmul K=120 -> PSUM f32)
#   ost       = copy(out)                              (ONE wide ACT copy -> fp16)
# This removes the PSUM->SBUF copy of au that the previous version needed
# (vector ops accept one PSUM operand, so the DVE multiply can consume au
# directly from PSUM). Per pair only 2 vector-engine ops run (DVE mul, ACT
# ocopy) instead of 3 per 500-atom tile before. All HBM IO is fp16.
#
# TRN2 notes driving the design:
#  - matmul output to PSUM must be fp32 (16-bit PSUM is TRN3+), so every
#    PSUM-source vector op runs at 1x: cost ~ (init + FD) cycles.
#  - PSUM = 8 banks x 2KB: au [120,1024] f32 = 2 banks x2 bufs, o [75,1024]
#    f32 = 2 banks x2 bufs -> exactly 8 banks, double-buffered.
#  - Wide (FD=1024, 2-bank contiguous) ops amortize the ~120-170 cycle init
#    + ~250ns dispatch overhead per instruction.
#
# Sharding: data-parallel over atoms, 75000 per core across 8 cores.

import numpy as np

N_ATOMS = 600000
NCORES = 8
NPC = N_ATOMS // NCORES   # 75000
PAIRW = 1024              # atoms per wide op (2 x 512-col PSUM halves)
NPAIRS = 74               # pairs per core
NPAD = NPAIRS * PAIRW     # 75776 padded atoms per core
CH = 2                    # pairs per input DMA chunk
OG = 2                    # pairs per output staging buffer / DMA

R = 8  # CP rank
BV40_MODE = False  # 40-row BV + on-chip SBUF->SBUF replicate measured 1.4x
                   # SLOWER than shipping all 120 rows (replication DMA is
                   # the bottleneck) -- ship the full 120-row BV_E.

U = np.array([[0.2419016152442985, 0.6625062831986197, -0.8309374270990885, 0.3998142823675103, -0.5651140448972596, -0.34640840162110975, 0.7646485241540064, -0.0981640650113134], [0.9679329076741274, -0.6672684032643771, -0.5353370910241713, -0.9127024843358726, 0.26799289625560263, 0.8715541794335616, -0.5278177753574712, -0.018552310924435454], [0.06774581008230969, 0.3403502647675755, 0.1515163067782647, -0.08439617705843598, 0.7802729803193187, 0.34697915153247866, 0.3697580702645849, -0.9949973005490104]])
V = np.array([[0.0026140108173807915, 0.6944345633371292, -0.5652773041221544, -0.35343275859595025, -0.03433664562735461, 0.08091670140460634, -0.0892103404240648, -0.1980300231087587], [0.2576248520364635, 0.06539948454957029, -0.35434557927644844, -0.03640441158856663, -0.7413593971475833, 0.0030001701455498278, 0.3713639451526768, 0.016947075929799594], [-0.5377309758940755, -0.02096760544900235, 0.40365084423895436, 0.5095417434602116, -0.45423293309175394, -0.5702820721334585, 0.6190313285414931, 0.7858326418298565], [0.7170730175523563, 0.7001885499108222, 0.4925926570601597, -0.7743826610421906, -0.16559112080190702, 0.6571136713106263, -0.6611900442465742, -0.2983796128216165], [0.36093529561820403, -0.15093011216763902, -0.38641849081949886, 0.1202443758222842, -0.4641758957921707, -0.4862339638412094, 0.1837342512310362, 0.5039182198056593]])
W = np.array([[0.7951356712114984, -0.07784905999497176, 0.08450253790371903, 0.006843070854248517, 0.2048617974624018, -1.523924051439455, 0.8830139483275325, 0.5211882387254724], [0.5093941381116157, -0.7659769028241413, -0.3653038243879763, -0.8496149079844891, 0.052715213787387104, 0.18251310702150852, 0.268561851999145, 0.9142889507799132], [0.021385010903070902, -0.4182776710107811, 0.26977388961992294, -1.1442626505742266, -1.0048448949104412, 0.34663597211489194, 1.2092826345430325, 0.8086175923533013], [-0.9015995943490751, 1.249123426342828, -0.5049639898080718, 2.545125440023137, 0.16782025096354364, -1.5011481522860137, 0.409842324079843, 0.27493076503176855], [0.9934580335307789, -0.10023212966102599, -0.4889278808326145, -2.6183798202363553, -0.4522780676075401, 1.1697194808175109, 0.8428489593111734, 0.2161166285673376]])


def _cp_factors_for(C):
    """Return (U, V, W) float64 with C[m,n,p] ~= sum_r U[m,r]V[n,r]W[p,r].

    Uses the embedded factors when C matches their reconstruction (the fixed
    real-CG tensor for l1=1, l2=2, L=2); otherwise fits a rank-8 CP
    decomposition to the given C at runtime via ALS with restarts.
    """
    C = np.asarray(C, dtype=np.float64)
    recon = np.einsum('mr,nr,pr->mnp', U, V, W)
    if np.abs(recon - C).max() < 1e-5 * max(1.0, np.abs(C).max()):
        return U, V, W

    def khatri(X, Y):
        return (X[:, None, :] * Y[None, :, :]).reshape(-1, X.shape[1])

    C1 = C.reshape(3, 25)
    C2 = C.transpose(1, 0, 2).reshape(5, 15)
    C3 = C.transpose(2, 0, 1).reshape(5, 15)
    best = None
    for seed in range(64):
        rng = np.random.default_rng(seed)
        u = rng.standard_normal((3, R))
        v = rng.standard_normal((5, R))
        w = rng.standard_normal((5, R))
        for _ in range(3000):
            u = C1 @ np.linalg.pinv(khatri(v, w).T)
            v = C2 @ np.linalg.pinv(khatri(u, w).T)
            w = C3 @ np.linalg.pinv(khatri(u, v).T)
        err = np.abs(np.einsum('mr,nr,pr->mnp', u, v, w) - C).max()
        if best is None or err < best[0]:
            best = (err, u, v, w)
        if err < 1e-9 * max(1.0, np.abs(C).max()):
            break
    err, u, v, w = best
    if err > 1e-5 * max(1.0, np.abs(C).max()):
        raise RuntimeError(f"runtime CP fit of C failed: absmax err {err}")
    su = np.linalg.norm(u, axis=0)
    sv = np.linalg.norm(v, axis=0)
    return u / su, v / sv, w * (su * sv)


def _build_weights(u, v, w):
    """WA [9,120], WB [25,120], WO [120,75] f32; q = a*40 + r*5 + b.

    a-block ordering: the three 40-row a-blocks of the (r,b) expansion are
    contiguous and identical in the B-side operand, so WB[:, :40] is the
    full information content (enables shipping 40 rows + on-chip replicate).
    """
    WA = np.zeros((9, 15 * R), np.float32)
    WB = np.zeros((25, 15 * R), np.float32)
    WO = np.zeros((15 * R, 75), np.float32)
    for r in range(R):
        for a in range(3):
            for b in range(5):
                q = a * 40 + r * 5 + b
                for m in range(3):
                    WA[m * 3 + a, q] = u[m, r]
                for n in range(5):
                    WB[n * 5 + b, q] = v[n, r]
                for p in range(5):
                    WO[q, p * 15 + a * 5 + b] = w[p, r]
    return WA, WB, WO


def _build_nc(WA, WO, reps=1, p_bufs=3, ost_bufs=2, ch=None, og=None,
              a_bufs=3, bv_bufs=3, bv40=BV40_MODE, pf=3):
    ch = CH if ch is None else ch
    og = OG if og is None else og
    if bv40:
        a_bufs = max(a_bufs, 2 + pf)
        bv_bufs = max(bv_bufs, 2 + pf)
    import concourse.bass as bass
    import concourse.bacc as bacc
    import concourse.mybir as mybir
    from concourse import tile

    f32 = mybir.dt.float32
    f16 = mybir.dt.float16

    nc = bacc.Bacc()
    a_in = nc.declare_dram_parameter("a_pack", [9, NPAD], f16, isOutput=False)
    bv_rows = 40 if bv40 else 15 * R
    bv_in = nc.declare_dram_parameter("bv_pack", [bv_rows, NPAD], f16,
                                      isOutput=False)
    out_d = nc.declare_dram_parameter("out_t", [75, NPAD], f16, isOutput=True)
    wa_d = nc.inline_tensor(WA.astype(np.float16), name="wa")
    wo_d = nc.inline_tensor(WO.astype(np.float16), name="wo")

    with tile.TileContext(nc) as tc:
        with (
            tc.tile_pool(name="const", bufs=1) as cpool,
            tc.tile_pool(name="a", bufs=a_bufs) as a_pool,
            tc.tile_pool(name="bv", bufs=bv_bufs) as bv_pool,
            tc.tile_pool(name="p", bufs=p_bufs) as p_pool,
            tc.tile_pool(name="ost", bufs=ost_bufs) as ost_pool,
            tc.tile_pool(name="au_ps", bufs=2, space=bass.MemorySpace.PSUM) as au_ps,
            tc.tile_pool(name="o_ps", bufs=2, space=bass.MemorySpace.PSUM) as o_ps,
        ):
            wa_t = cpool.tile([9, 15 * R], f16, tag="wa")
            wo_t = cpool.tile([15 * R, 75], f16, tag="wo")
            nc.gpsimd.dma_start(wa_t[:], wa_d[:, :])
            nc.gpsimd.dma_start(wo_t[:], wo_d[:, :])

            import contextlib
            rep_ctx = (tc.For_i(0, reps, 1) if reps > 1
                       else contextlib.nullcontext())
            with rep_ctx:
                # Software-pipelined by one pair: issue pair t's au-matmuls
                # BEFORE pair t-1's o-matmuls so the (in-order) PE queue
                # computes au(t) while the DVE multiply of pair t-1 runs.
                # Without this the PE's o-mm(t-1) [which waits on mul(t-1)]
                # blocks au-mm(t), serializing DVE and PE each pair.
                ost = None
                prev = None
                nchunks = (NPAIRS + ch - 1) // ch
                chunk_cache = {}

                def load_chunk(k):
                    cw = min(ch, NPAIRS - k * ch) * PAIRW
                    c0 = k * ch * PAIRW
                    a_t = a_pool.tile([9, cw], f16, tag="a")
                    nc.sync.dma_start(a_t[:], a_in[:, c0:c0 + cw])
                    bv_t = bv_pool.tile([15 * R, cw], f16, tag="bv")
                    if bv40:
                        # ship the 40 distinct (r,b) rows over HBM, build
                        # the 3 identical a-blocks with on-chip SBUF->SBUF
                        # DMA; pf-deep chunk prefetch hides the chain latency
                        nc.sync.dma_start(bv_t[0:40],
                                          bv_in[:, c0:c0 + cw])
                        nc.sync.dma_start(bv_t[40:80], bv_t[0:40])
                        nc.sync.dma_start(bv_t[80:120], bv_t[0:40])
                    else:
                        nc.sync.dma_start(bv_t[:], bv_in[:, c0:c0 + cw])
                    chunk_cache[k] = (a_t, bv_t)

                next_load = 0

                def load_through(k):
                    nonlocal next_load
                    while next_load <= min(k, nchunks - 1):
                        load_chunk(next_load)
                        next_load += 1

                for t in range(NPAIRS + 1):
                    if t < NPAIRS:
                        k, j = divmod(t, ch)
                        # prefetch pf chunks ahead of use
                        if j == 0 or t == 0:
                            load_through(k + pf)
                        a_t, bv_t = chunk_cache[k]
                        au = au_ps.tile([15 * R, PAIRW], f32, tag="au")
                        for h in (0, 1):
                            nc.tensor.matmul(
                                au[:, 512 * h:512 * (h + 1)],
                                wa_t[:],
                                a_t[:, j * PAIRW + 512 * h:
                                    j * PAIRW + 512 * (h + 1)],
                                tile_position=(0, 0),
                            )
                        cur = (au, bv_t, j)
                    else:
                        cur = None

                    if prev is not None:
                        au_p, bv_p, jp = prev
                        tp = t - 1
                        p = p_pool.tile([15 * R, PAIRW], f16, tag="p")
                        nc.vector.tensor_mul(
                            p[:], bv_p[:, jp * PAIRW:(jp + 1) * PAIRW],
                            au_p[:])
                        o = o_ps.tile([75, PAIRW], f32, tag="o")
                        for h in (0, 1):
                            nc.tensor.matmul(
                                o[:, 512 * h:512 * (h + 1)],
                                wo_t[:],
                                p[:, 512 * h:512 * (h + 1)],
                                tile_position=(0, 0),
                            )
                        g, gs = divmod(tp, og)
                        gw = min(og, NPAIRS - g * og)
                        if gs == 0:
                            ost = ost_pool.tile([75, gw * PAIRW], f16,
                                                tag="ost")
                        nc.scalar.copy(
                            ost[:, gs * PAIRW:(gs + 1) * PAIRW], o[:])
                        if gs == gw - 1:
                            # NOTE: issuing this on nc.scalar's HWDGE ring
                            # measured 163us vs 113.8us -- the trigger stalls
                            # the ACT ocopy stream. Keep it on nc.sync.
                            nc.sync.dma_start(
                                out_d[:, og * PAIRW * g:
                                      og * PAIRW * g + gw * PAIRW],
                                ost[:],
                            )
                    prev = cur
    nc.finalize()
    return nc


def _pack_inputs(A, B, WB, bv40=BV40_MODE):
    """Per-core packed fp16 [9, NPAD] (A) and [rows, NPAD] (BV) arrays."""
    rows = 40 if bv40 else 15 * R
    a_maps = []
    bv_maps = []
    BV = (B.reshape(N_ATOMS, 25) @ WB[:, :rows]).astype(np.float16)
    A16 = A.reshape(N_ATOMS, 9).astype(np.float16)
    for c in range(NCORES):
        Apack = np.zeros((9, NPAD), np.float16)
        Apack[:, :NPC] = A16[c * NPC:(c + 1) * NPC].T
        BVpack = np.zeros((rows, NPAD), np.float16)
        BVpack[:, :NPC] = BV[c * NPC:(c + 1) * NPC].T
        a_maps.append(Apack)
        bv_maps.append(BVpack)
    return a_maps, bv_maps


_NC_CACHE = {}


def kernel(A, B, C):
    from concourse.bass_utils import run_bass_kernel_spmd

    A = np.ascontiguousarray(np.asarray(A, dtype=np.float32))
    B = np.ascontiguousarray(np.asarray(B, dtype=np.float32))
    C = np.asarray(C, dtype=np.float32)

    key = C.tobytes()
    if key not in _NC_CACHE:
        u, v, w = _cp_factors_for(C)
        WA, WB, WO = _build_weights(u, v, w)
        _NC_CACHE[key] = (_build_nc(WA, WO), WB)
    nc, WB = _NC_CACHE[key]

    a_maps, bv_maps = _pack_inputs(A, B, WB)
    in_maps = [{"a_pack": a_maps[c], "bv_pack": bv_maps[c]}
               for c in range(NCORES)]
    res = run_bass_kernel_spmd(nc, in_maps, list(range(NCORES)))
    outs = [res.results[c]["out_t"][:, :NPC] for c in range(NCORES)]
    full = np.concatenate(outs, axis=1).astype(np.float32)  # [75, 600000]
    return np.ascontiguousarray(full.T).reshape(N_ATOMS, 5, 15)


if __name__ == "__main__":
    rng = np.random.default_rng(0)
    A = rng.standard_normal((N_ATOMS, 3, 3)).astype(np.float32)
    B = rng.standard_normal((N_ATOMS, 5, 5)).astype(np.float32)
    C = np.einsum('mr,nr,pr->mnp', U, V, W).astype(np.float32)
    out = kernel(A, B, C)
    print(out.shape, out.dtype)
